# revision 2
# baseline (speedup 1.0000x reference)
"""Dense dot-product attention on 8 Trainium2 NeuronCores — v3.

Problem: query/key/value [32, 2048, 64] fp32 -> softmax(Q K^T / 8) V.
Sharding: batch split 4-per-core across 8 cores, no collectives.

Per core, per batch:
  1. Q,K loaded fp32, DVE-cast to fp16 with d-columns DUPLICATED into
     [128, 16, 128] (halves identical), then xbar DMA-transpose
     SBUF->SBUF into qhT/khT [128 dd, 16, 128] (= [dd, 2048 seq]).
     No PE transposes.
  2. S^T via 2-way row-packed matmul pairs (tile_position (0,0)/(64,0)):
     k-tiles 2kp, 2kp+1 concurrently into s_pair [128, 2, 512] PSUM
     (2 banks; triple-buffered). HW-verified ~110ns/MM when streamed.
  3. exp(s/8 - 2) split across TWO engines running concurrently
     (slots emitted in groups of 2, QK two slots ahead):
     - ScalarE slots: true exp -> fp16 e-planes.
     - DVE slots: Schraudolph int16-bitcast exp -> fp16 (max err ~3%).
     All-fp16 (fp8 PV was tried: concourse float8e4 is IEEE e4m3 with
     max-normal 240 -> Inf, and 3-bit-mantissa p/V quantization puts
     worst-case output error at ~2.9% > the 2e-2 budget).
  4. PV: per k-tile fp16 matmul accumulating out^T [65, 512] in PSUM
     (col 64 = softmax denominator via ones column in V weights).
  5. ScalarE evacuates pv -> pv_sb [80, 2048] fp16; one xbar transpose
     per batch -> o_nat [128, 16, 80]; DVE reciprocal + broadcast mul
     -> fp32 out, DMA store.
"""

import numpy as np

B, L, D = 32, 2048, 64
NCORES = 8
B_SH = B // NCORES          # 4 batches per core
LT = L // 128               # 16 k tiles
NKP = LT // 2               # 8 k-tile pairs
NQH = 4                     # q chunks
QHW = L // NQH              # 512
MPAD = 80                   # pv_sb padded rows (64 V + 1 den + 15 pad)
SCALE = 1.0 / np.sqrt(np.float32(D))  # 0.125
LN2 = float(np.log(2.0))
EXP_BIAS = -2.0             # exp(s*SCALE + EXP_BIAS); cancels in softmax
A_SCH = (1024.0 / LN2) * float(SCALE)
B_SCH = 15315.5 + (1024.0 / LN2) * EXP_BIAS

# per-qh tuple: which kp slots run exp on DVE (rest on ScalarE): 13/32
DVE_KP = {0: (1, 3, 5), 1: (2, 4, 6), 2: (1, 3, 5, 7), 3: (2, 4, 6)}

_cached = {}


def _build():
    import concourse.bacc as bacc
    import concourse.tile as tile
    from concourse import mybir

    f32 = mybir.dt.float32
    f16 = mybir.dt.float16
    i16 = mybir.dt.int16
    Exp = mybir.ActivationFunctionType.Exp
    Alu = mybir.AluOpType

    nc = bacc.Bacc("TRN2", target_bir_lowering=False, debug=False)

    q_d = nc.dram_tensor("query", [B_SH, L, D], f32, kind="ExternalInput")
    k_d = nc.dram_tensor("key", [B_SH, L, D], f32, kind="ExternalInput")
    v_d = nc.dram_tensor("value", [B_SH, L, D], f32, kind="ExternalInput")
    o_d = nc.dram_tensor("out", [B_SH, L, D], f32, kind="ExternalOutput")

    with tile.TileContext(nc) as tc:
        with (
            tc.tile_pool(name="consts", bufs=1) as consts,
            tc.tile_pool(name="nat", bufs=2) as nat,
            tc.tile_pool(name="dup", bufs=2) as dupp,
            tc.tile_pool(name="qkt", bufs=2) as qkt,
            tc.tile_pool(name="vst", bufs=2) as vst,
            tc.tile_pool(name="vr", bufs=2) as vrp,
            tc.tile_pool(name="e16", bufs=4) as e16p,
            tc.tile_pool(name="pvsb", bufs=2) as pvsbp,
            tc.tile_pool(name="onat", bufs=2) as onatp,
            tc.tile_pool(name="ofin", bufs=2) as ofinp,
            tc.tile_pool(name="rz", bufs=2) as rzp,
            tc.tile_pool(name="sps", bufs=3, space="PSUM") as sps,
            tc.tile_pool(name="pvps", bufs=2, space="PSUM") as pvps,
        ):
            wsrc = consts.tile([128, 512], f16)
            nc.vector.memset(wsrc, 0.001)
            dummy = consts.tile([128, 1], f32)
            nc.vector.memset(dummy, 0.0)
            nc.scalar.activation(out=dummy, in_=dummy, func=Exp, scale=1.0)
            bias_t = consts.tile([128, 1], f32)
            nc.vector.memset(bias_t, EXP_BIAS)

            def warmer():
                wt = sps.tile([64, 512], f32, tag="s")
                nc.tensor.matmul(wt, wsrc[:, 0:64], wsrc,
                                 start=True, stop=True, skip_group_check=True)

            qkT = {}
            v_r = {}

            def prep_load(b):
                """jobs: DMA loads + casts(+dup) + xbar transposes."""
                q_nat = nat.tile([128, LT, D], f32, tag="qnat")
                k_nat = nat.tile([128, LT, D], f32, tag="knat")
                q_r = q_d.ap()[b].rearrange("(t p) d -> p t d", p=128)
                k_r = k_d.ap()[b].rearrange("(t p) d -> p t d", p=128)
                nc.sync.dma_start(out=k_nat[:, 0:8, :], in_=k_r[:, 0:8, :])
                nc.sync.dma_start(out=q_nat[:, 0:4, :], in_=q_r[:, 0:4, :])
                nc.sync.dma_start(out=k_nat[:, 8:LT, :], in_=k_r[:, 8:LT, :])
                nc.sync.dma_start(out=q_nat[:, 4:LT, :], in_=q_r[:, 4:LT, :])

                qd = dupp.tile([128, LT, 128], f16, tag="qd")
                kd = dupp.tile([128, LT, 128], f16, tag="kd")
                qhT = qkt.tile([128, LT, 128], f16, tag="qhT")
                khT = qkt.tile([128, LT, 128], f16, tag="khT")

                v_stage = vst.tile([128, LT, D], f32, tag="vstage")
                nc.sync.dma_start(
                    out=v_stage, in_=v_d.ap()[b].rearrange("(t p) d -> p t d", p=128))
                vr16 = vrp.tile([128, LT, D + 1], f16, tag="vr16")

                jobs = []

                def cast_job(dst, src, lo, hi, half):
                    def job():
                        nc.vector.tensor_copy(
                            out=dst[:, lo:hi, half * D:(half + 1) * D],
                            in_=src[:, lo:hi, :])
                    return job

                def xbar_job(dst, src, lo, hi):
                    def job():
                        nc.sync.dma_start_transpose(
                            dst[:, lo:hi, :], src[:, lo:hi, :])
                    return job

                # K first (qh=0 consumes all k tiles), then V, then Q
                for lo in range(0, LT, 4):
                    jobs.append(cast_job(kd, k_nat, lo, lo + 4, 0))
                    jobs.append(cast_job(kd, k_nat, lo, lo + 4, 1))
                    jobs.append(xbar_job(khT, kd, lo, lo + 4))

                def v16_job():
                    nc.vector.tensor_copy(out=vr16[:, :, 0:D], in_=v_stage)
                    nc.vector.memset(vr16[:, :, D:D + 1], 1.0)

                jobs.append(v16_job)
                for lo in range(0, LT, 4):
                    jobs.append(cast_job(qd, q_nat, lo, lo + 4, 0))
                    jobs.append(cast_job(qd, q_nat, lo, lo + 4, 1))
                    jobs.append(xbar_job(qhT, qd, lo, lo + 4))

                qkT[b] = (qhT, khT)
                v_r[b] = vr16
                return jobs

            from collections import deque
            work = deque()     # phase-2 closures, delayed ~2 slots
            prep_jobs = deque()

            def emit_one():
                if work:
                    work.popleft()()

            def main(b, nxt):
                # all of THIS batch's prep must already be emitted: readers
                # bind to the last write emitted before them.
                while prep_jobs:
                    prep_jobs.popleft()()
                qhT, khT = qkT.pop(b)
                vr16 = v_r.pop(b)
                qhT2 = qhT.rearrange("p t q -> p (t q)")
                khT2 = khT.rearrange("p t q -> p (t q)")
                pv_sb = pvsbp.tile([MPAD, L], f16, tag="pvsb")
                prep_jobs.extend(nxt)

                for qh in range(NQH):
                    qs = slice(qh * QHW, (qh + 1) * QHW)
                    pv = pvps.tile([D + 1, QHW], f32, tag="pv")
                    for kp in range(NKP):
                        ka, kb = 2 * kp, 2 * kp + 1
                        s_pair = sps.tile([128, 2, QHW], f32, tag="s")
                        nc.tensor.matmul(
                            s_pair[:, 0, :],
                            khT2[0:64, ka * 128:(ka + 1) * 128],
                            qhT2[0:64, qs], start=True, stop=True,
                            tile_position=(0, 0))
                        nc.tensor.matmul(
                            s_pair[:, 1, :],
                            khT2[64:128, kb * 128:(kb + 1) * 128],
                            qhT2[64:128, qs], start=True, stop=True,
                            tile_position=(64, 0))

                        def phase2(s_pair=s_pair, pv=pv, ka=ka, kb=kb,
                                   kp=kp, use_dve=kp in DVE_KP[qh]):
                            e16 = e16p.tile([128, 2, QHW], f16, tag="e16")
                            if use_dve:
                                nc.vector.tensor_scalar(
                                    out=e16.bitcast(i16), in0=s_pair,
                                    scalar1=A_SCH, scalar2=B_SCH,
                                    op0=Alu.mult, op1=Alu.add)
                            else:
                                nc.scalar.activation(
                                    out=e16, in_=s_pair, func=Exp,
                                    scale=float(SCALE), bias=bias_t)
                            nc.tensor.matmul(
                                pv, vr16[:, ka, :], e16[:, 0, :],
                                start=(kp == 0), stop=False,
                                skip_group_check=True)
                            nc.tensor.matmul(
                                pv, vr16[:, kb, :], e16[:, 1, :],
                                start=False, stop=(kp == NKP - 1),
                                skip_group_check=True)
                        emit_one()
                        work.append(phase2)
                        if prep_jobs:
                            prep_jobs.popleft()()

                    def evac(pv=pv, qs=qs):
                        nc.scalar.copy(out=pv_sb[0:D + 1, qs], in_=pv)
                    work.append(evac)

                def out_job(b=b, pv_sb=pv_sb):
                    o_nat = onatp.tile([128, LT, MPAD], f16, tag="onat")
                    nc.sync.dma_start_transpose(o_nat, pv_sb)
                    rz = rzp.tile([128, LT], f32, tag="rz")
                    nc.vector.reciprocal(out=rz, in_=o_nat[:, :, D])
                    o_fin = ofinp.tile([128, LT, D], f32, tag="ofin")
                    nc.vector.tensor_tensor(
                        out=o_fin, in0=o_nat[:, :, 0:D],
                        in1=rz.unsqueeze(2).broadcast_to([128, LT, D]),
                        op=Alu.mult)
                    nc.sync.dma_start(
                        out=o_d.ap()[b].rearrange("(t p) d -> p t d", p=128),
                        in_=o_fin)
                work.append(out_job)

            for _ in range(12):
                warmer()
            jobs0 = prep_load(0)
            for job in jobs0:
                job()
            for b in range(B_SH):
                nxt = prep_load(b + 1) if b + 1 < B_SH else []
                main(b, nxt)
            while work:
                emit_one()
            while prep_jobs:
                prep_jobs.popleft()()

    nc.finalize()
    return nc


def _get_nc():
    if "nc" not in _cached:
        _cached["nc"] = _build()
    return _cached["nc"]


def kernel(query, key, value):
    from concourse.bass_utils import run_bass_kernel_spmd

    nc = _get_nc()
    query = np.ascontiguousarray(query, dtype=np.float32)
    key = np.ascontiguousarray(key, dtype=np.float32)
    value = np.ascontiguousarray(value, dtype=np.float32)

    in_maps = []
    for c in range(NCORES):
        sl = slice(c * B_SH, (c + 1) * B_SH)
        in_maps.append({
            "query": query[sl], "key": key[sl], "value": value[sl]})

    res = run_bass_kernel_spmd(nc, in_maps, core_ids=list(range(NCORES)))
    out = np.concatenate([r["out"] for r in res.results], axis=0)
    return out


# revision 3
# speedup vs baseline: 1.0085x; 1.0085x over previous
"""Dense dot-product attention on 8 Trainium2 NeuronCores — v3.

Problem: query/key/value [32, 2048, 64] fp32 -> softmax(Q K^T / 8) V.
Sharding: batch split 4-per-core across 8 cores, no collectives.

Per core, per batch:
  1. Q,K loaded fp32, DVE-cast to fp16 with d-columns DUPLICATED into
     [128, 16, 128] (halves identical), then xbar DMA-transpose
     SBUF->SBUF into qhT/khT [128 dd, 16, 128] (= [dd, 2048 seq]).
     No PE transposes.
  2. S^T via 2-way row-packed matmul pairs (tile_position (0,0)/(64,0)):
     k-tiles 2kp, 2kp+1 concurrently into s_pair [128, 2, 512] PSUM
     (2 banks; triple-buffered). HW-verified ~110ns/MM when streamed.
  3. exp(s/8 - 2) split across TWO engines running concurrently
     (slots emitted in groups of 2, QK two slots ahead):
     - ScalarE slots: true exp -> fp16 e-planes.
     - DVE slots: Schraudolph int16-bitcast exp -> fp16 (max err ~3%).
     All-fp16 (fp8 PV was tried: concourse float8e4 is IEEE e4m3 with
     max-normal 240 -> Inf, and 3-bit-mantissa p/V quantization puts
     worst-case output error at ~2.9% > the 2e-2 budget).
  4. PV: per k-tile fp16 matmul accumulating out^T [65, 512] in PSUM
     (col 64 = softmax denominator via ones column in V weights).
  5. ScalarE evacuates pv -> pv_sb [80, 2048] fp16; one xbar transpose
     per batch -> o_nat [128, 16, 80]; DVE reciprocal + broadcast mul
     -> fp32 out, DMA store.
"""

import numpy as np

B, L, D = 32, 2048, 64
NCORES = 8
B_SH = B // NCORES          # 4 batches per core
LT = L // 128               # 16 k tiles
NKP = LT // 2               # 8 k-tile pairs
NQH = 4                     # q chunks
QHW = L // NQH              # 512
MPAD = 80                   # pv_sb padded rows (64 V + 1 den + 15 pad)
SCALE = 1.0 / np.sqrt(np.float32(D))  # 0.125
LN2 = float(np.log(2.0))
EXP_BIAS = -2.0             # exp(s*SCALE + EXP_BIAS); cancels in softmax
A_SCH = (1024.0 / LN2) * float(SCALE)
B_SCH = 15315.5 + (1024.0 / LN2) * EXP_BIAS

# per-qh tuple: which kp slots run exp on DVE (rest on ScalarE): 13/32
DVE_KP = {0: (1, 3, 5), 1: (2, 4, 6), 2: (1, 3, 5, 7), 3: (2, 4, 6)}

_cached = {}


def _build():
    import concourse.bacc as bacc
    import concourse.tile as tile
    from concourse import mybir

    f32 = mybir.dt.float32
    f16 = mybir.dt.float16
    i16 = mybir.dt.int16
    Exp = mybir.ActivationFunctionType.Exp
    Alu = mybir.AluOpType

    nc = bacc.Bacc("TRN2", target_bir_lowering=False, debug=False)

    q_d = nc.dram_tensor("query", [B_SH, L, D], f32, kind="ExternalInput")
    k_d = nc.dram_tensor("key", [B_SH, L, D], f32, kind="ExternalInput")
    v_d = nc.dram_tensor("value", [B_SH, L, D], f32, kind="ExternalInput")
    o_d = nc.dram_tensor("out", [B_SH, L, D], f32, kind="ExternalOutput")

    with tile.TileContext(nc) as tc:
        with (
            tc.tile_pool(name="consts", bufs=1) as consts,
            tc.tile_pool(name="nat", bufs=2) as nat,
            tc.tile_pool(name="dup", bufs=2) as dupp,
            tc.tile_pool(name="qkt", bufs=2) as qkt,
            tc.tile_pool(name="vst", bufs=2) as vst,
            tc.tile_pool(name="vr", bufs=2) as vrp,
            tc.tile_pool(name="e16", bufs=4) as e16p,
            tc.tile_pool(name="pvsb", bufs=2) as pvsbp,
            tc.tile_pool(name="onat", bufs=2) as onatp,
            tc.tile_pool(name="ofin", bufs=2) as ofinp,
            tc.tile_pool(name="rz", bufs=2) as rzp,
            tc.tile_pool(name="sps", bufs=3, space="PSUM") as sps,
            tc.tile_pool(name="pvps", bufs=2, space="PSUM") as pvps,
        ):
            wsrc = consts.tile([128, 512], f16)
            nc.vector.memset(wsrc, 0.001)
            dummy = consts.tile([128, 1], f32)
            nc.vector.memset(dummy, 0.0)
            nc.scalar.activation(out=dummy, in_=dummy, func=Exp, scale=1.0)
            bias_t = consts.tile([128, 1], f32)
            nc.vector.memset(bias_t, EXP_BIAS)

            def warmer():
                wt = sps.tile([64, 512], f32, tag="s")
                nc.tensor.matmul(wt, wsrc[:, 0:64], wsrc,
                                 start=True, stop=True, skip_group_check=True)

            qkT = {}
            v_r = {}

            def prep_load(b):
                """jobs: DMA loads + casts(+dup) + xbar transposes."""
                q_nat = nat.tile([128, LT, D], f32, tag="qnat")
                k_nat = nat.tile([128, LT, D], f32, tag="knat")
                q_r = q_d.ap()[b].rearrange("(t p) d -> p t d", p=128)
                k_r = k_d.ap()[b].rearrange("(t p) d -> p t d", p=128)
                nc.sync.dma_start(out=k_nat[:, 0:8, :], in_=k_r[:, 0:8, :])
                nc.sync.dma_start(out=q_nat[:, 0:4, :], in_=q_r[:, 0:4, :])
                nc.sync.dma_start(out=k_nat[:, 8:LT, :], in_=k_r[:, 8:LT, :])
                nc.sync.dma_start(out=q_nat[:, 4:LT, :], in_=q_r[:, 4:LT, :])

                qd = dupp.tile([128, LT, 128], f16, tag="qd")
                kd = dupp.tile([128, LT, 128], f16, tag="kd")
                qhT = qkt.tile([128, LT, 128], f16, tag="qhT")
                khT = qkt.tile([128, LT, 128], f16, tag="khT")

                v_stage = vst.tile([128, LT, D], f32, tag="vstage")
                nc.sync.dma_start(
                    out=v_stage, in_=v_d.ap()[b].rearrange("(t p) d -> p t d", p=128))
                vr16 = vrp.tile([128, LT, D + 1], f16, tag="vr16")

                jobs = []

                def cast_job(dst, src, lo, hi, half):
                    def job():
                        nc.vector.tensor_copy(
                            out=dst[:, lo:hi, half * D:(half + 1) * D],
                            in_=src[:, lo:hi, :])
                    return job

                def xbar_job(dst, src, lo, hi):
                    def job():
                        nc.sync.dma_start_transpose(
                            dst[:, lo:hi, :], src[:, lo:hi, :])
                    return job

                # K first (qh=0 consumes all k tiles), then V, then Q
                for lo in range(0, LT, 4):
                    jobs.append(cast_job(kd, k_nat, lo, lo + 4, 0))
                    jobs.append(cast_job(kd, k_nat, lo, lo + 4, 1))
                    jobs.append(xbar_job(khT, kd, lo, lo + 4))

                def v16_job():
                    nc.vector.tensor_copy(out=vr16[:, :, 0:D], in_=v_stage)
                    nc.vector.memset(vr16[:, :, D:D + 1], 1.0)

                jobs.append(v16_job)
                for lo in range(0, LT, 4):
                    jobs.append(cast_job(qd, q_nat, lo, lo + 4, 0))
                    jobs.append(cast_job(qd, q_nat, lo, lo + 4, 1))
                    jobs.append(xbar_job(qhT, qd, lo, lo + 4))

                qkT[b] = (qhT, khT)
                v_r[b] = vr16
                return jobs

            from collections import deque
            work = deque()     # phase-2 closures, delayed ~2 slots
            prep_jobs = deque()

            def emit_one():
                if work:
                    work.popleft()()

            def main(b, nxt):
                # all of THIS batch's prep must already be emitted: readers
                # bind to the last write emitted before them.
                while prep_jobs:
                    prep_jobs.popleft()()
                qhT, khT = qkT.pop(b)
                vr16 = v_r.pop(b)
                qhT2 = qhT.rearrange("p t q -> p (t q)")
                khT2 = khT.rearrange("p t q -> p (t q)")
                pv_sb = pvsbp.tile([MPAD, L], f16, tag="pvsb")
                prep_jobs.extend(nxt)

                for qh in range(NQH):
                    qs = slice(qh * QHW, (qh + 1) * QHW)
                    pv = pvps.tile([D + 1, QHW], f32, tag="pv")
                    for kp in range(NKP):
                        ka, kb = 2 * kp, 2 * kp + 1
                        s_pair = sps.tile([128, 2, QHW], f32, tag="s")
                        nc.tensor.matmul(
                            s_pair[:, 0, :],
                            khT2[0:64, ka * 128:(ka + 1) * 128],
                            qhT2[0:64, qs], start=True, stop=True,
                            tile_position=(0, 0))
                        nc.tensor.matmul(
                            s_pair[:, 1, :],
                            khT2[64:128, kb * 128:(kb + 1) * 128],
                            qhT2[64:128, qs], start=True, stop=True,
                            tile_position=(64, 0))

                        def phase2(s_pair=s_pair, pv=pv, ka=ka, kb=kb,
                                   kp=kp, use_dve=kp in DVE_KP[qh]):
                            e16 = e16p.tile([128, 2, QHW], f16, tag="e16")
                            if use_dve:
                                nc.vector.tensor_scalar(
                                    out=e16.bitcast(i16), in0=s_pair,
                                    scalar1=A_SCH, scalar2=B_SCH,
                                    op0=Alu.mult, op1=Alu.add)
                            else:
                                nc.scalar.activation(
                                    out=e16, in_=s_pair, func=Exp,
                                    scale=float(SCALE), bias=bias_t)
                            nc.tensor.matmul(
                                pv, vr16[:, ka, :], e16[:, 0, :],
                                start=(kp == 0), stop=False,
                                skip_group_check=True)
                            nc.tensor.matmul(
                                pv, vr16[:, kb, :], e16[:, 1, :],
                                start=False, stop=(kp == NKP - 1),
                                skip_group_check=True)
                        emit_one()
                        work.append(phase2)
                        if prep_jobs:
                            prep_jobs.popleft()()
                        if prep_jobs:
                            prep_jobs.popleft()()

                    def evac(pv=pv, qs=qs):
                        nc.scalar.copy(out=pv_sb[0:D + 1, qs], in_=pv)
                    work.append(evac)

                def out_job(b=b, pv_sb=pv_sb):
                    o_nat = onatp.tile([128, LT, MPAD], f16, tag="onat")
                    nc.sync.dma_start_transpose(o_nat, pv_sb)
                    rz = rzp.tile([128, LT], f32, tag="rz")
                    nc.vector.reciprocal(out=rz, in_=o_nat[:, :, D])
                    o_fin = ofinp.tile([128, LT, D], f32, tag="ofin")
                    nc.vector.tensor_tensor(
                        out=o_fin, in0=o_nat[:, :, 0:D],
                        in1=rz.unsqueeze(2).broadcast_to([128, LT, D]),
                        op=Alu.mult)
                    nc.sync.dma_start(
                        out=o_d.ap()[b].rearrange("(t p) d -> p t d", p=128),
                        in_=o_fin)
                work.append(out_job)

            for _ in range(12):
                warmer()
            jobs0 = prep_load(0)
            for job in jobs0:
                job()
            for b in range(B_SH):
                nxt = prep_load(b + 1) if b + 1 < B_SH else []
                main(b, nxt)
            while work:
                emit_one()
            while prep_jobs:
                prep_jobs.popleft()()

    nc.finalize()
    return nc


def _get_nc():
    if "nc" not in _cached:
        _cached["nc"] = _build()
    return _cached["nc"]


def kernel(query, key, value):
    from concourse.bass_utils import run_bass_kernel_spmd

    nc = _get_nc()
    query = np.ascontiguousarray(query, dtype=np.float32)
    key = np.ascontiguousarray(key, dtype=np.float32)
    value = np.ascontiguousarray(value, dtype=np.float32)

    in_maps = []
    for c in range(NCORES):
        sl = slice(c * B_SH, (c + 1) * B_SH)
        in_maps.append({
            "query": query[sl], "key": key[sl], "value": value[sl]})

    res = run_bass_kernel_spmd(nc, in_maps, core_ids=list(range(NCORES)))
    out = np.concatenate([r["out"] for r in res.results], axis=0)
    return out


# revision 4
# speedup vs baseline: 1.0357x; 1.0270x over previous
"""Dense dot-product attention on 8 Trainium2 NeuronCores — v3.

Problem: query/key/value [32, 2048, 64] fp32 -> softmax(Q K^T / 8) V.
Sharding: batch split 4-per-core across 8 cores, no collectives.

Per core, per batch:
  1. Q,K loaded fp32, DVE-cast to fp16 with d-columns DUPLICATED into
     [128, 16, 128] (halves identical), then xbar DMA-transpose
     SBUF->SBUF into qhT/khT [128 dd, 16, 128] (= [dd, 2048 seq]).
     No PE transposes.
  2. S^T via 2-way row-packed matmul pairs (tile_position (0,0)/(64,0)):
     k-tiles 2kp, 2kp+1 concurrently into s_pair [128, 2, 512] PSUM
     (2 banks; triple-buffered). HW-verified ~110ns/MM when streamed.
  3. exp(s/8 - 2) split across TWO engines running concurrently
     (slots emitted in groups of 2, QK two slots ahead):
     - ScalarE slots: true exp -> fp16 e-planes.
     - DVE slots: Schraudolph int16-bitcast exp -> fp16 (max err ~3%).
     All-fp16 (fp8 PV was tried: concourse float8e4 is IEEE e4m3 with
     max-normal 240 -> Inf, and 3-bit-mantissa p/V quantization puts
     worst-case output error at ~2.9% > the 2e-2 budget).
  4. PV: per k-tile fp16 matmul accumulating out^T [65, 512] in PSUM
     (col 64 = softmax denominator via ones column in V weights).
  5. ScalarE evacuates pv -> pv_sb [80, 2048] fp16; one xbar transpose
     per batch -> o_nat [128, 16, 80]; DVE reciprocal + broadcast mul
     -> fp32 out, DMA store.
"""

import numpy as np

B, L, D = 32, 2048, 64
NCORES = 8
B_SH = B // NCORES          # 4 batches per core
LT = L // 128               # 16 k tiles
NKP = LT // 2               # 8 k-tile pairs
NQH = 4                     # q chunks
QHW = L // NQH              # 512
MPAD = 80                   # pv_sb padded rows (64 V + 1 den + 15 pad)
SCALE = 1.0 / np.sqrt(np.float32(D))  # 0.125
LN2 = float(np.log(2.0))
EXP_BIAS = -2.0             # exp(s*SCALE + EXP_BIAS); cancels in softmax
A_SCH = (1024.0 / LN2) * float(SCALE)
B_SCH = 15315.5 + (1024.0 / LN2) * EXP_BIAS

# per-qh tuple: which kp slots run exp on DVE (rest on ScalarE): 13/32
DVE_KP = {0: (1, 3, 5), 1: (2, 4, 6), 2: (1, 3, 5, 7), 3: (2, 4, 6)}

_cached = {}


def _build():
    import concourse.bacc as bacc
    import concourse.tile as tile
    from concourse import mybir

    f32 = mybir.dt.float32
    f16 = mybir.dt.float16
    i16 = mybir.dt.int16
    Exp = mybir.ActivationFunctionType.Exp
    Alu = mybir.AluOpType

    nc = bacc.Bacc("TRN2", target_bir_lowering=False, debug=False)

    q_d = nc.dram_tensor("query", [B_SH, L, D], f32, kind="ExternalInput")
    k_d = nc.dram_tensor("key", [B_SH, L, D], f32, kind="ExternalInput")
    v_d = nc.dram_tensor("value", [B_SH, L, D], f32, kind="ExternalInput")
    o_d = nc.dram_tensor("out", [B_SH, L, D], f32, kind="ExternalOutput")

    with tile.TileContext(nc) as tc:
        with (
            tc.tile_pool(name="consts", bufs=1) as consts,
            tc.tile_pool(name="nat", bufs=3) as nat,
            tc.tile_pool(name="dup", bufs=2) as dupp,
            tc.tile_pool(name="qkt", bufs=2) as qkt,
            tc.tile_pool(name="vst", bufs=3) as vst,
            tc.tile_pool(name="vr", bufs=2) as vrp,
            tc.tile_pool(name="e16", bufs=4) as e16p,
            tc.tile_pool(name="pvsb", bufs=2) as pvsbp,
            tc.tile_pool(name="onat", bufs=2) as onatp,
            tc.tile_pool(name="ofin", bufs=2) as ofinp,
            tc.tile_pool(name="rz", bufs=2) as rzp,
            tc.tile_pool(name="sps", bufs=3, space="PSUM") as sps,
            tc.tile_pool(name="pvps", bufs=2, space="PSUM") as pvps,
        ):
            wsrc = consts.tile([128, 512], f16)
            nc.vector.memset(wsrc, 0.001)
            dummy = consts.tile([128, 1], f32)
            nc.vector.memset(dummy, 0.0)
            nc.scalar.activation(out=dummy, in_=dummy, func=Exp, scale=1.0)
            bias_t = consts.tile([128, 1], f32)
            nc.vector.memset(bias_t, EXP_BIAS)

            def warmer():
                wt = sps.tile([64, 512], f32, tag="s")
                nc.tensor.matmul(wt, wsrc[:, 0:64], wsrc,
                                 start=True, stop=True, skip_group_check=True)

            qkT = {}
            v_r = {}

            staged = {}

            def prep_dispatch(b):
                """issue batch b's input loads a half-batch early."""
                q_nat = nat.tile([128, LT, D], f32, tag="qnat")
                k_nat = nat.tile([128, LT, D], f32, tag="knat")
                v_stage = vst.tile([128, LT, D], f32, tag="vstage")
                q_r = q_d.ap()[b].rearrange("(t p) d -> p t d", p=128)
                k_r = k_d.ap()[b].rearrange("(t p) d -> p t d", p=128)
                nc.sync.dma_start(out=k_nat[:, 0:8, :], in_=k_r[:, 0:8, :])
                nc.sync.dma_start(out=q_nat[:, 0:4, :], in_=q_r[:, 0:4, :])
                nc.sync.dma_start(out=k_nat[:, 8:LT, :], in_=k_r[:, 8:LT, :])
                nc.sync.dma_start(out=q_nat[:, 4:LT, :], in_=q_r[:, 4:LT, :])
                nc.sync.dma_start(
                    out=v_stage, in_=v_d.ap()[b].rearrange("(t p) d -> p t d", p=128))
                staged[b] = (q_nat, k_nat, v_stage)

            def prep_load(b):
                """jobs: casts(+dup) + xbar transposes for batch b."""
                q_nat, k_nat, v_stage = staged.pop(b)
                qd = dupp.tile([128, LT, 128], f16, tag="qd")
                kd = dupp.tile([128, LT, 128], f16, tag="kd")
                qhT = qkt.tile([128, LT, 128], f16, tag="qhT")
                khT = qkt.tile([128, LT, 128], f16, tag="khT")
                vr16 = vrp.tile([128, LT, D + 1], f16, tag="vr16")

                jobs = []

                def cast_job(dst, src, lo, hi, half):
                    def job():
                        nc.vector.tensor_copy(
                            out=dst[:, lo:hi, half * D:(half + 1) * D],
                            in_=src[:, lo:hi, :])
                    return job

                def xbar_job(dst, src, lo, hi):
                    def job():
                        nc.sync.dma_start_transpose(
                            dst[:, lo:hi, :], src[:, lo:hi, :])
                    return job

                # K first (qh=0 consumes all k tiles), then V, then Q
                for lo in range(0, LT, 4):
                    jobs.append(cast_job(kd, k_nat, lo, lo + 4, 0))
                    jobs.append(cast_job(kd, k_nat, lo, lo + 4, 1))
                    jobs.append(xbar_job(khT, kd, lo, lo + 4))

                def v16_job():
                    nc.vector.tensor_copy(out=vr16[:, :, 0:D], in_=v_stage)
                    nc.vector.memset(vr16[:, :, D:D + 1], 1.0)

                jobs.append(v16_job)
                for lo in range(0, LT, 4):
                    jobs.append(cast_job(qd, q_nat, lo, lo + 4, 0))
                    jobs.append(cast_job(qd, q_nat, lo, lo + 4, 1))
                    jobs.append(xbar_job(qhT, qd, lo, lo + 4))

                qkT[b] = (qhT, khT)
                v_r[b] = vr16
                return jobs

            from collections import deque
            work = deque()     # phase-2 closures, delayed ~2 slots
            prep_jobs = deque()

            def emit_one():
                if work:
                    work.popleft()()

            def main(b, nxt):
                # all of THIS batch's prep must already be emitted: readers
                # bind to the last write emitted before them.
                while prep_jobs:
                    prep_jobs.popleft()()
                qhT, khT = qkT.pop(b)
                vr16 = v_r.pop(b)
                qhT2 = qhT.rearrange("p t q -> p (t q)")
                khT2 = khT.rearrange("p t q -> p (t q)")
                pv_sb = pvsbp.tile([MPAD, L], f16, tag="pvsb")
                prep_jobs.extend(nxt)

                for qh in range(NQH):
                    qs = slice(qh * QHW, (qh + 1) * QHW)
                    pv = pvps.tile([D + 1, QHW], f32, tag="pv")
                    for kp in range(NKP):
                        ka, kb = 2 * kp, 2 * kp + 1
                        s_pair = sps.tile([128, 2, QHW], f32, tag="s")
                        nc.tensor.matmul(
                            s_pair[:, 0, :],
                            khT2[0:64, ka * 128:(ka + 1) * 128],
                            qhT2[0:64, qs], start=True, stop=True,
                            tile_position=(0, 0))
                        nc.tensor.matmul(
                            s_pair[:, 1, :],
                            khT2[64:128, kb * 128:(kb + 1) * 128],
                            qhT2[64:128, qs], start=True, stop=True,
                            tile_position=(64, 0))

                        def phase2(s_pair=s_pair, pv=pv, ka=ka, kb=kb,
                                   kp=kp, use_dve=kp in DVE_KP[qh]):
                            e16 = e16p.tile([128, 2, QHW], f16, tag="e16")
                            if use_dve:
                                nc.vector.tensor_scalar(
                                    out=e16.bitcast(i16), in0=s_pair,
                                    scalar1=A_SCH, scalar2=B_SCH,
                                    op0=Alu.mult, op1=Alu.add)
                            else:
                                nc.scalar.activation(
                                    out=e16, in_=s_pair, func=Exp,
                                    scale=float(SCALE), bias=bias_t)
                            nc.tensor.matmul(
                                pv, vr16[:, ka, :], e16[:, 0, :],
                                start=(kp == 0), stop=False,
                                skip_group_check=True)
                            nc.tensor.matmul(
                                pv, vr16[:, kb, :], e16[:, 1, :],
                                start=False, stop=(kp == NKP - 1),
                                skip_group_check=True)
                        emit_one()
                        work.append(phase2)
                        if prep_jobs:
                            prep_jobs.popleft()()
                        if prep_jobs:
                            prep_jobs.popleft()()
                        if qh == 2 and kp == 0 and b + 2 < B_SH:
                            prep_dispatch(b + 2)

                    def evac(pv=pv, qs=qs):
                        nc.scalar.copy(out=pv_sb[0:D + 1, qs], in_=pv)
                    work.append(evac)

                def out_job(b=b, pv_sb=pv_sb):
                    o_nat = onatp.tile([128, LT, MPAD], f16, tag="onat")
                    nc.sync.dma_start_transpose(o_nat, pv_sb)
                    rz = rzp.tile([128, LT], f32, tag="rz")
                    nc.vector.reciprocal(out=rz, in_=o_nat[:, :, D])
                    o_fin = ofinp.tile([128, LT, D], f32, tag="ofin")
                    nc.vector.tensor_tensor(
                        out=o_fin, in0=o_nat[:, :, 0:D],
                        in1=rz.unsqueeze(2).broadcast_to([128, LT, D]),
                        op=Alu.mult)
                    nc.sync.dma_start(
                        out=o_d.ap()[b].rearrange("(t p) d -> p t d", p=128),
                        in_=o_fin)
                work.append(out_job)

            prep_dispatch(0)
            for _ in range(12):
                warmer()
            jobs0 = prep_load(0)
            for job in jobs0:
                job()
            prep_dispatch(1)
            for b in range(B_SH):
                nxt = prep_load(b + 1) if b + 1 < B_SH else []
                main(b, nxt)
            while work:
                emit_one()
            while prep_jobs:
                prep_jobs.popleft()()

    nc.finalize()
    return nc


def _get_nc():
    if "nc" not in _cached:
        _cached["nc"] = _build()
    return _cached["nc"]


def kernel(query, key, value):
    from concourse.bass_utils import run_bass_kernel_spmd

    nc = _get_nc()
    query = np.ascontiguousarray(query, dtype=np.float32)
    key = np.ascontiguousarray(key, dtype=np.float32)
    value = np.ascontiguousarray(value, dtype=np.float32)

    in_maps = []
    for c in range(NCORES):
        sl = slice(c * B_SH, (c + 1) * B_SH)
        in_maps.append({
            "query": query[sl], "key": key[sl], "value": value[sl]})

    res = run_bass_kernel_spmd(nc, in_maps, core_ids=list(range(NCORES)))
    out = np.concatenate([r["out"] for r in res.results], axis=0)
    return out


# revision 5
# speedup vs baseline: 1.0513x; 1.0150x over previous
"""Dense dot-product attention on 8 Trainium2 NeuronCores — v3.

Problem: query/key/value [32, 2048, 64] fp32 -> softmax(Q K^T / 8) V.
Sharding: batch split 4-per-core across 8 cores, no collectives.

Per core, per batch:
  1. Q,K loaded fp32, DVE-cast to fp16 with d-columns DUPLICATED into
     [128, 16, 128] (halves identical), then xbar DMA-transpose
     SBUF->SBUF into qhT/khT [128 dd, 16, 128] (= [dd, 2048 seq]).
     No PE transposes.
  2. S^T via 2-way row-packed matmul pairs (tile_position (0,0)/(64,0)):
     k-tiles 2kp, 2kp+1 concurrently into s_pair [128, 2, 512] PSUM
     (2 banks; triple-buffered). HW-verified ~110ns/MM when streamed.
  3. exp(s/8 - 2) split across TWO engines running concurrently
     (slots emitted in groups of 2, QK two slots ahead):
     - ScalarE slots: true exp -> fp16 e-planes.
     - DVE slots: Schraudolph int16-bitcast exp -> fp16 (max err ~3%).
     All-fp16 (fp8 PV was tried: concourse float8e4 is IEEE e4m3 with
     max-normal 240 -> Inf, and 3-bit-mantissa p/V quantization puts
     worst-case output error at ~2.9% > the 2e-2 budget).
  4. PV: per k-tile fp16 matmul accumulating out^T [65, 512] in PSUM
     (col 64 = softmax denominator via ones column in V weights).
  5. ScalarE evacuates pv -> pv_sb [80, 2048] fp16; one xbar transpose
     per batch -> o_nat [128, 16, 80]; DVE reciprocal + broadcast mul
     -> fp32 out, DMA store.
"""

import numpy as np

B, L, D = 32, 2048, 64
NCORES = 8
B_SH = B // NCORES          # 4 batches per core
LT = L // 128               # 16 k tiles
NKP = LT // 2               # 8 k-tile pairs
NQH = 4                     # q chunks
QHW = L // NQH              # 512
MPAD = 80                   # pv_sb padded rows (64 V + 1 den + 15 pad)
SCALE = 1.0 / np.sqrt(np.float32(D))  # 0.125
LN2 = float(np.log(2.0))
EXP_BIAS = -2.0             # exp(s*SCALE + EXP_BIAS); cancels in softmax
A_SCH = (1024.0 / LN2) * float(SCALE)
B_SCH = 15315.5 + (1024.0 / LN2) * EXP_BIAS

# per-qh tuple: which kp slots run exp on DVE (rest on ScalarE): 13/32
DVE_KP = {0: (1, 3, 5), 1: (1, 3, 5, 7), 2: (1, 3, 5, 7), 3: (2, 4, 6)}

_cached = {}


def _build():
    import concourse.bacc as bacc
    import concourse.tile as tile
    from concourse import mybir

    f32 = mybir.dt.float32
    f16 = mybir.dt.float16
    i16 = mybir.dt.int16
    Exp = mybir.ActivationFunctionType.Exp
    Alu = mybir.AluOpType

    nc = bacc.Bacc("TRN2", target_bir_lowering=False, debug=False)

    q_d = nc.dram_tensor("query", [B_SH, L, D], f32, kind="ExternalInput")
    k_d = nc.dram_tensor("key", [B_SH, L, D], f32, kind="ExternalInput")
    v_d = nc.dram_tensor("value", [B_SH, L, D], f32, kind="ExternalInput")
    o_d = nc.dram_tensor("out", [B_SH, L, D], f32, kind="ExternalOutput")

    with tile.TileContext(nc) as tc:
        with (
            tc.tile_pool(name="consts", bufs=1) as consts,
            tc.tile_pool(name="nat", bufs=3) as nat,
            tc.tile_pool(name="dup", bufs=2) as dupp,
            tc.tile_pool(name="qkt", bufs=2) as qkt,
            tc.tile_pool(name="vst", bufs=3) as vst,
            tc.tile_pool(name="vr", bufs=2) as vrp,
            tc.tile_pool(name="e16", bufs=5) as e16p,
            tc.tile_pool(name="pvsb", bufs=2) as pvsbp,
            tc.tile_pool(name="onat", bufs=2) as onatp,
            tc.tile_pool(name="ofin", bufs=2) as ofinp,
            tc.tile_pool(name="rz", bufs=2) as rzp,
            tc.tile_pool(name="sps", bufs=3, space="PSUM") as sps,
            tc.tile_pool(name="pvps", bufs=2, space="PSUM") as pvps,
        ):
            wsrc = consts.tile([128, 512], f16)
            nc.vector.memset(wsrc, 0.001)
            dummy = consts.tile([128, 1], f32)
            nc.vector.memset(dummy, 0.0)
            nc.scalar.activation(out=dummy, in_=dummy, func=Exp, scale=1.0)
            bias_t = consts.tile([128, 1], f32)
            nc.vector.memset(bias_t, EXP_BIAS)

            def warmer():
                wt = sps.tile([64, 512], f32, tag="s")
                nc.tensor.matmul(wt, wsrc[:, 0:64], wsrc,
                                 start=True, stop=True, skip_group_check=True)

            qkT = {}
            v_r = {}

            staged = {}

            def prep_dispatch(b):
                """issue batch b's input loads a half-batch early."""
                q_nat = nat.tile([128, LT, D], f32, tag="qnat")
                k_nat = nat.tile([128, LT, D], f32, tag="knat")
                v_stage = vst.tile([128, LT, D], f32, tag="vstage")
                q_r = q_d.ap()[b].rearrange("(t p) d -> p t d", p=128)
                k_r = k_d.ap()[b].rearrange("(t p) d -> p t d", p=128)
                nc.sync.dma_start(out=k_nat[:, 0:8, :], in_=k_r[:, 0:8, :])
                nc.sync.dma_start(out=q_nat[:, 0:4, :], in_=q_r[:, 0:4, :])
                nc.sync.dma_start(out=k_nat[:, 8:LT, :], in_=k_r[:, 8:LT, :])
                nc.sync.dma_start(out=q_nat[:, 4:LT, :], in_=q_r[:, 4:LT, :])
                nc.sync.dma_start(
                    out=v_stage, in_=v_d.ap()[b].rearrange("(t p) d -> p t d", p=128))
                staged[b] = (q_nat, k_nat, v_stage)

            def prep_load(b):
                """jobs: casts(+dup) + xbar transposes for batch b."""
                q_nat, k_nat, v_stage = staged.pop(b)
                qd = dupp.tile([128, LT, 128], f16, tag="qd")
                kd = dupp.tile([128, LT, 128], f16, tag="kd")
                qhT = qkt.tile([128, LT, 128], f16, tag="qhT")
                khT = qkt.tile([128, LT, 128], f16, tag="khT")
                vr16 = vrp.tile([128, LT, D + 1], f16, tag="vr16")

                jobs = []

                def cast_job(dst, src, lo, hi, half):
                    def job():
                        nc.vector.tensor_copy(
                            out=dst[:, lo:hi, half * D:(half + 1) * D],
                            in_=src[:, lo:hi, :])
                    return job

                def xbar_job(dst, src, lo, hi):
                    def job():
                        nc.sync.dma_start_transpose(
                            dst[:, lo:hi, :], src[:, lo:hi, :])
                    return job

                def v16_job():
                    nc.vector.tensor_copy(out=vr16[:, :, 0:D], in_=v_stage)
                    nc.vector.memset(vr16[:, :, D:D + 1], 1.0)

                # first k AND q quarters first (slot-0 critical path), then
                # rest of K (qh=0 consumes all k tiles), V, rest of Q
                for d_, s_, t_ in ((kd, k_nat, khT), (qd, q_nat, qhT)):
                    jobs.append(cast_job(d_, s_, 0, 4, 0))
                    jobs.append(cast_job(d_, s_, 0, 4, 1))
                    jobs.append(xbar_job(t_, d_, 0, 4))
                for lo in range(4, LT, 4):
                    jobs.append(cast_job(kd, k_nat, lo, lo + 4, 0))
                    jobs.append(cast_job(kd, k_nat, lo, lo + 4, 1))
                    jobs.append(xbar_job(khT, kd, lo, lo + 4))
                jobs.append(v16_job)
                for lo in range(4, LT, 4):
                    jobs.append(cast_job(qd, q_nat, lo, lo + 4, 0))
                    jobs.append(cast_job(qd, q_nat, lo, lo + 4, 1))
                    jobs.append(xbar_job(qhT, qd, lo, lo + 4))

                qkT[b] = (qhT, khT)
                v_r[b] = vr16
                return jobs

            from collections import deque
            work = deque()     # phase-2 closures, delayed ~2 slots
            prep_jobs = deque()

            def emit_one():
                if work:
                    work.popleft()()

            def main(b, nxt):
                # all of THIS batch's prep must already be emitted: readers
                # bind to the last write emitted before them.
                while prep_jobs:
                    prep_jobs.popleft()()
                qhT, khT = qkT.pop(b)
                vr16 = v_r.pop(b)
                qhT2 = qhT.rearrange("p t q -> p (t q)")
                khT2 = khT.rearrange("p t q -> p (t q)")
                pv_sb = pvsbp.tile([MPAD, L], f16, tag="pvsb")
                prep_jobs.extend(nxt)

                for qh in range(NQH):
                    qs = slice(qh * QHW, (qh + 1) * QHW)
                    pv = pvps.tile([D + 1, QHW], f32, tag="pv")
                    for kp in range(NKP):
                        ka, kb = 2 * kp, 2 * kp + 1
                        s_pair = sps.tile([128, 2, QHW], f32, tag="s")
                        nc.tensor.matmul(
                            s_pair[:, 0, :],
                            khT2[0:64, ka * 128:(ka + 1) * 128],
                            qhT2[0:64, qs], start=True, stop=True,
                            tile_position=(0, 0))
                        nc.tensor.matmul(
                            s_pair[:, 1, :],
                            khT2[64:128, kb * 128:(kb + 1) * 128],
                            qhT2[64:128, qs], start=True, stop=True,
                            tile_position=(64, 0))

                        def phase2(s_pair=s_pair, pv=pv, ka=ka, kb=kb,
                                   kp=kp, use_dve=kp in DVE_KP[qh]):
                            e16 = e16p.tile([128, 2, QHW], f16, tag="e16")
                            if use_dve:
                                nc.vector.tensor_scalar(
                                    out=e16.bitcast(i16), in0=s_pair,
                                    scalar1=A_SCH, scalar2=B_SCH,
                                    op0=Alu.mult, op1=Alu.add)
                            else:
                                nc.scalar.activation(
                                    out=e16, in_=s_pair, func=Exp,
                                    scale=float(SCALE), bias=bias_t)
                            nc.tensor.matmul(
                                pv, vr16[:, ka, :], e16[:, 0, :],
                                start=(kp == 0), stop=False,
                                skip_group_check=True)
                            nc.tensor.matmul(
                                pv, vr16[:, kb, :], e16[:, 1, :],
                                start=False, stop=(kp == NKP - 1),
                                skip_group_check=True)
                        emit_one()
                        work.append(phase2)
                        if prep_jobs:
                            prep_jobs.popleft()()
                        if prep_jobs:
                            prep_jobs.popleft()()
                        if qh == 2 and kp == 0 and b + 2 < B_SH:
                            prep_dispatch(b + 2)

                    def evac(pv=pv, qs=qs):
                        nc.scalar.copy(out=pv_sb[0:D + 1, qs], in_=pv)
                    work.append(evac)

                def out_job(b=b, pv_sb=pv_sb):
                    o_nat = onatp.tile([128, LT, MPAD], f16, tag="onat")
                    nc.sync.dma_start_transpose(o_nat, pv_sb)
                    rz = rzp.tile([128, LT], f32, tag="rz")
                    nc.vector.reciprocal(out=rz, in_=o_nat[:, :, D])
                    o_fin = ofinp.tile([128, LT, D], f32, tag="ofin")
                    nc.vector.tensor_tensor(
                        out=o_fin, in0=o_nat[:, :, 0:D],
                        in1=rz.unsqueeze(2).broadcast_to([128, LT, D]),
                        op=Alu.mult)
                    nc.sync.dma_start(
                        out=o_d.ap()[b].rearrange("(t p) d -> p t d", p=128),
                        in_=o_fin)
                work.append(out_job)

            prep_dispatch(0)
            for _ in range(9):
                warmer()
            jobs0 = prep_load(0)
            for job in jobs0:
                job()
            prep_dispatch(1)
            for b in range(B_SH):
                nxt = prep_load(b + 1) if b + 1 < B_SH else []
                main(b, nxt)
            while work:
                emit_one()
            while prep_jobs:
                prep_jobs.popleft()()

    nc.finalize()
    return nc


def _get_nc():
    if "nc" not in _cached:
        _cached["nc"] = _build()
    return _cached["nc"]


def kernel(query, key, value):
    from concourse.bass_utils import run_bass_kernel_spmd

    nc = _get_nc()
    query = np.ascontiguousarray(query, dtype=np.float32)
    key = np.ascontiguousarray(key, dtype=np.float32)
    value = np.ascontiguousarray(value, dtype=np.float32)

    in_maps = []
    for c in range(NCORES):
        sl = slice(c * B_SH, (c + 1) * B_SH)
        in_maps.append({
            "query": query[sl], "key": key[sl], "value": value[sl]})

    res = run_bass_kernel_spmd(nc, in_maps, core_ids=list(range(NCORES)))
    out = np.concatenate([r["out"] for r in res.results], axis=0)
    return out


# revision 6
# speedup vs baseline: 1.0565x; 1.0050x over previous
"""Dense dot-product attention on 8 Trainium2 NeuronCores — v3.

Problem: query/key/value [32, 2048, 64] fp32 -> softmax(Q K^T / 8) V.
Sharding: batch split 4-per-core across 8 cores, no collectives.

Per core, per batch:
  1. Q,K loaded fp32, DVE-cast to fp16 with d-columns DUPLICATED into
     [128, 16, 128] (halves identical), then xbar DMA-transpose
     SBUF->SBUF into qhT/khT [128 dd, 16, 128] (= [dd, 2048 seq]).
     No PE transposes.
  2. S^T via 2-way row-packed matmul pairs (tile_position (0,0)/(64,0)):
     k-tiles 2kp, 2kp+1 concurrently into s_pair [128, 2, 512] PSUM
     (2 banks; triple-buffered). HW-verified ~110ns/MM when streamed.
  3. exp(s/8 - 2) split across TWO engines running concurrently
     (slots emitted in groups of 2, QK two slots ahead):
     - ScalarE slots: true exp -> fp16 e-planes.
     - DVE slots: Schraudolph int16-bitcast exp -> fp16 (max err ~3%).
     All-fp16 (fp8 PV was tried: concourse float8e4 is IEEE e4m3 with
     max-normal 240 -> Inf, and 3-bit-mantissa p/V quantization puts
     worst-case output error at ~2.9% > the 2e-2 budget).
  4. PV: per k-tile fp16 matmul accumulating out^T [65, 512] in PSUM
     (col 64 = softmax denominator via ones column in V weights).
  5. ScalarE evacuates pv -> pv_sb [80, 2048] fp16; one xbar transpose
     per batch -> o_nat [128, 16, 80]; DVE reciprocal + broadcast mul
     -> fp32 out, DMA store.
"""

import numpy as np

B, L, D = 32, 2048, 64
NCORES = 8
B_SH = B // NCORES          # 4 batches per core
LT = L // 128               # 16 k tiles
NKP = LT // 2               # 8 k-tile pairs
NQH = 4                     # q chunks
QHW = L // NQH              # 512
MPAD = 80                   # pv_sb padded rows (64 V + 1 den + 15 pad)
SCALE = 1.0 / np.sqrt(np.float32(D))  # 0.125
LN2 = float(np.log(2.0))
EXP_BIAS = -2.0             # exp(s*SCALE + EXP_BIAS); cancels in softmax
A_SCH = (1024.0 / LN2) * float(SCALE)
B_SCH = 15315.5 + (1024.0 / LN2) * EXP_BIAS

# per-qh tuple: which kp slots run exp on DVE (rest on ScalarE): 13/32
DVE_KP = {0: (1, 3, 5), 1: (1, 3, 5, 7), 2: (1, 3, 5, 7), 3: (2, 4, 6)}

_cached = {}


def _build():
    import concourse.bacc as bacc
    import concourse.tile as tile
    from concourse import mybir

    f32 = mybir.dt.float32
    f16 = mybir.dt.float16
    i16 = mybir.dt.int16
    Exp = mybir.ActivationFunctionType.Exp
    Alu = mybir.AluOpType

    nc = bacc.Bacc("TRN2", target_bir_lowering=False, debug=False)

    q_d = nc.dram_tensor("query", [B_SH, L, D], f32, kind="ExternalInput")
    k_d = nc.dram_tensor("key", [B_SH, L, D], f32, kind="ExternalInput")
    v_d = nc.dram_tensor("value", [B_SH, L, D], f32, kind="ExternalInput")
    o_d = nc.dram_tensor("out", [B_SH, L, D], f32, kind="ExternalOutput")

    with tile.TileContext(nc) as tc:
        with (
            tc.tile_pool(name="consts", bufs=1) as consts,
            tc.tile_pool(name="nat", bufs=3) as nat,
            tc.tile_pool(name="dup", bufs=2) as dupp,
            tc.tile_pool(name="qkt", bufs=2) as qkt,
            tc.tile_pool(name="vst", bufs=3) as vst,
            tc.tile_pool(name="vr", bufs=2) as vrp,
            tc.tile_pool(name="e16", bufs=5) as e16p,
            tc.tile_pool(name="pvsb", bufs=2) as pvsbp,
            tc.tile_pool(name="onat", bufs=2) as onatp,
            tc.tile_pool(name="ofin", bufs=2) as ofinp,
            tc.tile_pool(name="rz", bufs=2) as rzp,
            tc.tile_pool(name="sps", bufs=3, space="PSUM") as sps,
            tc.tile_pool(name="pvps", bufs=2, space="PSUM") as pvps,
        ):
            wsrc = consts.tile([128, 512], f16)
            nc.vector.memset(wsrc, 0.001)
            dummy = consts.tile([128, 1], f32)
            nc.vector.memset(dummy, 0.0)
            nc.scalar.activation(out=dummy, in_=dummy, func=Exp, scale=1.0)
            bias_t = consts.tile([128, 1], f32)
            nc.vector.memset(bias_t, EXP_BIAS)

            def warmer():
                wt = sps.tile([64, 512], f32, tag="s")
                nc.tensor.matmul(wt, wsrc[:, 0:64], wsrc,
                                 start=True, stop=True, skip_group_check=True)

            qkT = {}
            v_r = {}

            staged = {}

            def prep_dispatch(b):
                """issue batch b's input loads a half-batch early."""
                q_nat = nat.tile([128, LT, D], f32, tag="qnat")
                k_nat = nat.tile([128, LT, D], f32, tag="knat")
                v_stage = vst.tile([128, LT, D], f32, tag="vstage")
                q_r = q_d.ap()[b].rearrange("(t p) d -> p t d", p=128)
                k_r = k_d.ap()[b].rearrange("(t p) d -> p t d", p=128)
                nc.sync.dma_start(out=k_nat[:, 0:8, :], in_=k_r[:, 0:8, :])
                nc.sync.dma_start(out=q_nat[:, 0:4, :], in_=q_r[:, 0:4, :])
                nc.sync.dma_start(out=k_nat[:, 8:LT, :], in_=k_r[:, 8:LT, :])
                nc.sync.dma_start(out=q_nat[:, 4:LT, :], in_=q_r[:, 4:LT, :])
                nc.sync.dma_start(
                    out=v_stage, in_=v_d.ap()[b].rearrange("(t p) d -> p t d", p=128))
                staged[b] = (q_nat, k_nat, v_stage)

            def prep_load(b):
                """jobs: casts(+dup) + xbar transposes for batch b."""
                q_nat, k_nat, v_stage = staged.pop(b)
                qd = dupp.tile([128, LT, 128], f16, tag="qd")
                kd = dupp.tile([128, LT, 128], f16, tag="kd")
                qhT = qkt.tile([128, LT, 128], f16, tag="qhT")
                khT = qkt.tile([128, LT, 128], f16, tag="khT")
                vr16 = vrp.tile([128, LT, D + 1], f16, tag="vr16")

                jobs = []

                def cast_job(dst, src, lo, hi, half):
                    def job():
                        nc.vector.tensor_copy(
                            out=dst[:, lo:hi, half * D:(half + 1) * D],
                            in_=src[:, lo:hi, :])
                    return job

                def xbar_job(dst, src, lo, hi):
                    def job():
                        nc.sync.dma_start_transpose(
                            dst[:, lo:hi, :], src[:, lo:hi, :])
                    return job

                def v16_job():
                    nc.vector.tensor_copy(out=vr16[:, :, 0:D], in_=v_stage)
                    nc.vector.memset(vr16[:, :, D:D + 1], 1.0)

                # first k AND q quarters first (slot-0 critical path), then
                # rest of K (qh=0 consumes all k tiles), V, rest of Q
                for d_, s_, t_ in ((kd, k_nat, khT), (qd, q_nat, qhT)):
                    jobs.append(cast_job(d_, s_, 0, 4, 0))
                    jobs.append(cast_job(d_, s_, 0, 4, 1))
                    jobs.append(xbar_job(t_, d_, 0, 4))
                for lo in range(4, LT, 4):
                    jobs.append(cast_job(kd, k_nat, lo, lo + 4, 0))
                    jobs.append(cast_job(kd, k_nat, lo, lo + 4, 1))
                    jobs.append(xbar_job(khT, kd, lo, lo + 4))
                jobs.append(v16_job)
                for lo in range(4, LT, 4):
                    jobs.append(cast_job(qd, q_nat, lo, lo + 4, 0))
                    jobs.append(cast_job(qd, q_nat, lo, lo + 4, 1))
                    jobs.append(xbar_job(qhT, qd, lo, lo + 4))

                qkT[b] = (qhT, khT)
                v_r[b] = vr16
                return jobs

            from collections import deque
            work = deque()     # phase-2 closures, delayed ~2 slots
            prep_jobs = deque()

            def emit_one():
                if work:
                    work.popleft()()

            def main(b, nxt):
                # all of THIS batch's prep must already be emitted: readers
                # bind to the last write emitted before them.
                while prep_jobs:
                    prep_jobs.popleft()()
                qhT, khT = qkT.pop(b)
                vr16 = v_r.pop(b)
                qhT2 = qhT.rearrange("p t q -> p (t q)")
                khT2 = khT.rearrange("p t q -> p (t q)")
                pv_sb = pvsbp.tile([MPAD, L], f16, tag="pvsb")
                prep_jobs.extend(nxt)

                for qh in range(NQH):
                    qs = slice(qh * QHW, (qh + 1) * QHW)
                    pv = pvps.tile([D + 1, QHW], f32, tag="pv")
                    for kp in range(NKP):
                        ka, kb = 2 * kp, 2 * kp + 1
                        s_pair = sps.tile([128, 2, QHW], f32, tag="s")
                        nc.tensor.matmul(
                            s_pair[:, 0, :],
                            khT2[0:64, ka * 128:(ka + 1) * 128],
                            qhT2[0:64, qs], start=True, stop=True,
                            tile_position=(0, 0))
                        nc.tensor.matmul(
                            s_pair[:, 1, :],
                            khT2[64:128, kb * 128:(kb + 1) * 128],
                            qhT2[64:128, qs], start=True, stop=True,
                            tile_position=(64, 0))

                        def phase2(s_pair=s_pair, pv=pv, ka=ka, kb=kb,
                                   kp=kp, use_dve=kp in DVE_KP[qh]):
                            e16 = e16p.tile([128, 2, QHW], f16, tag="e16")
                            if use_dve:
                                nc.vector.tensor_scalar(
                                    out=e16.bitcast(i16), in0=s_pair,
                                    scalar1=A_SCH, scalar2=B_SCH,
                                    op0=Alu.mult, op1=Alu.add)
                            else:
                                nc.scalar.activation(
                                    out=e16, in_=s_pair, func=Exp,
                                    scale=float(SCALE), bias=bias_t)
                            nc.tensor.matmul(
                                pv, vr16[:, ka, :], e16[:, 0, :],
                                start=(kp == 0), stop=False,
                                skip_group_check=True)
                            nc.tensor.matmul(
                                pv, vr16[:, kb, :], e16[:, 1, :],
                                start=False, stop=(kp == NKP - 1),
                                skip_group_check=True)
                        emit_one()
                        work.append(phase2)
                        if prep_jobs:
                            prep_jobs.popleft()()
                        if prep_jobs:
                            prep_jobs.popleft()()
                        if qh == 2 and kp == 0 and b + 2 < B_SH:
                            prep_dispatch(b + 2)

                    def evac(pv=pv, qs=qs):
                        nc.scalar.copy(out=pv_sb[0:D + 1, qs], in_=pv)
                    work.append(evac)

                    # last batch: drain output in halves so the tail chain
                    # overlaps the remaining compute
                    if b == B_SH - 1 and qh in (1, 3):
                        def out_half(b=b, pv_sb=pv_sb, hb=qh // 2):
                            HT = LT // 2
                            o_nat = onatp.tile([128, HT, MPAD], f16, tag="onat")
                            nc.sync.dma_start_transpose(
                                o_nat, pv_sb[:, hb * 1024:(hb + 1) * 1024])
                            rz = rzp.tile([128, HT], f32, tag="rz")
                            nc.vector.reciprocal(out=rz, in_=o_nat[:, :, D])
                            o_fin = ofinp.tile([128, HT, D], f32, tag="ofin")
                            nc.vector.tensor_tensor(
                                out=o_fin, in0=o_nat[:, :, 0:D],
                                in1=rz.unsqueeze(2).broadcast_to([128, HT, D]),
                                op=Alu.mult)
                            nc.sync.dma_start(
                                out=o_d.ap()[b, hb * 1024:(hb + 1) * 1024, :]
                                    .rearrange("(t p) d -> p t d", p=128),
                                in_=o_fin)
                        work.append(out_half)

                if b == B_SH - 1:
                    return

                def out_job(b=b, pv_sb=pv_sb):
                    o_nat = onatp.tile([128, LT, MPAD], f16, tag="onat")
                    nc.sync.dma_start_transpose(o_nat, pv_sb)
                    rz = rzp.tile([128, LT], f32, tag="rz")
                    nc.vector.reciprocal(out=rz, in_=o_nat[:, :, D])
                    o_fin = ofinp.tile([128, LT, D], f32, tag="ofin")
                    nc.vector.tensor_tensor(
                        out=o_fin, in0=o_nat[:, :, 0:D],
                        in1=rz.unsqueeze(2).broadcast_to([128, LT, D]),
                        op=Alu.mult)
                    nc.sync.dma_start(
                        out=o_d.ap()[b].rearrange("(t p) d -> p t d", p=128),
                        in_=o_fin)
                work.append(out_job)

            prep_dispatch(0)
            for _ in range(9):
                warmer()
            jobs0 = prep_load(0)
            for job in jobs0:
                job()
            prep_dispatch(1)
            for b in range(B_SH):
                nxt = prep_load(b + 1) if b + 1 < B_SH else []
                main(b, nxt)
            while work:
                emit_one()
            while prep_jobs:
                prep_jobs.popleft()()

    nc.finalize()
    return nc


def _get_nc():
    if "nc" not in _cached:
        _cached["nc"] = _build()
    return _cached["nc"]


def kernel(query, key, value):
    from concourse.bass_utils import run_bass_kernel_spmd

    nc = _get_nc()
    query = np.ascontiguousarray(query, dtype=np.float32)
    key = np.ascontiguousarray(key, dtype=np.float32)
    value = np.ascontiguousarray(value, dtype=np.float32)

    in_maps = []
    for c in range(NCORES):
        sl = slice(c * B_SH, (c + 1) * B_SH)
        in_maps.append({
            "query": query[sl], "key": key[sl], "value": value[sl]})

    res = run_bass_kernel_spmd(nc, in_maps, core_ids=list(range(NCORES)))
    out = np.concatenate([r["out"] for r in res.results], axis=0)
    return out
